# revision 21
# baseline (speedup 1.0000x reference)
"""MiniSTU (spectral transform unit) Trainium2 kernel — fp8 DoubleRow version.

Math: out[b,l,o] = sum_k sum_{t<=l} phi_k[l-t] * (x[b,t] @ C_k^{par(l-t)})[o]
where C^even = Mp+Mm =: A, C^odd = Mp-Mm =: B  (parity merge of the two
sign branches of the reference's FFT conv — halves the conv work).

Deinterleaving t and l by parity turns this into 4 half-length (512) causal
block-Toeplitz convolutions built from three filter families:
  famE[d]  = phi[2d]   (A-stream, both output parities)
  famO1[d] = phi[2d-1] (B-stream -> even outputs)
  famO2[d] = phi[2d+1] (B-stream -> odd outputs)

All matmuls run as fp8e4 (e4m3) DoubleRow (0.5 cycles/row in the cost
model) with 3-term error compensation: for X@W with X1=q(X), X2=q(X-X1),
W1=q(W), W2=q(W-W1):  X@W ~= X1W1 + X2W1 + X1W2  (dropped X2W2 ~ 0.1%).
The DoubleRow pair dim carries the two i-chunks (projection: contraction
256 per instruction) or the two streams A/B (conv: both streams' block
contributions per instruction).

Sharding: K=24 filters split 3-per-core across 8 cores; host sums the 8
partial outputs.  Scales (powers of 2): x*8, M*8, W*256; device psum
y = 64*y_true, out_psum = 16384*out_true; the 1/16384 is applied by the
on-device output copy.
"""

import os
os.environ.setdefault("NEURON_RT_RESET_CORES", "1")

import numpy as np
import ml_dtypes
import concourse.bacc as bacc
import concourse.mybir as mybir
from concourse.tile import TileContext
import concourse.bass as bass_mod
from concourse.bass_utils import run_bass_kernel_spmd

B, L, I, O, K = 4, 1024, 256, 256, 24
S = 128
NAB = 4            # deinterleaved half-length 512 = 4 blocks of 128
KPC = 3
N_CORES = 8
F32 = mybir.dt.float32
F8 = mybir.dt.float8e4
F8NP = ml_dtypes.float8_e4m3
BF16 = mybir.dt.bfloat16
DR = mybir.MatmulPerfMode.DoubleRow

SX, SM, SW = 8.0, 8.0, 256.0
OUT_SCALE = 1.0 / (SX * SM * SW)

_cache = {}


def _build_program():
    nc = bacc.Bacc()
    # xt[term][i_lo, ic*4096 + b*1024 + ab*256 + par*128 + a]
    xt_d = nc.declare_dram_parameter("xt", [2, S, 8192], F8, isOutput=False)
    # mc[term][i_lo, ic*1536 + s*768 + kp*256 + o]
    mc_d = nc.declare_dram_parameter("mc", [2, S, 3072], F8, isOutput=False)
    # w[term][po, d, br, s*384 + kp*128 + ar]
    w_d = nc.declare_dram_parameter("w", [2, 2, NAB, S, 768], F8, isOutput=False)
    # out[po, ab, oh, ar, b*128+o]
    out_d = nc.declare_dram_parameter("out", [2, NAB, 2, S, B * S], F32,
                                      isOutput=True)

    with TileContext(nc) as tc:
        with tc.tile_pool(name="persist", bufs=1) as persist, \
             tc.tile_pool(name="ostage", bufs=3) as ostage, \
             tc.tile_pool(name="pyp", bufs=2, space="PSUM") as pyp, \
             tc.tile_pool(name="poutp", bufs=2, space="PSUM") as poutp:

            # ---- persistent SBUF tiles ----
            # xt tile per (term, b): cols = ic*1024 + ab*256 + par*128 + a
            xt_sb = {}
            for t in range(2):
                for b in range(B):
                    xt_sb[t, b] = persist.tile([S, 2048], F8, tag=f"xt{t}{b}",
                                               name=f"xt_sb{t}{b}")
            mc_sb = {}
            for t in range(2):
                mc_sb[t] = persist.tile([S, 3072], F8, tag=f"mc{t}",
                                        name=f"mc_sb{t}")
            w_sb = {}
            for t in range(2):
                for po in range(2):
                    for d in range(NAB):
                        w_sb[t, po, d] = persist.tile(
                            [S, 768], F8, tag=f"w{t}{po}{d}",
                            name=f"w_sb{t}{po}{d}")
            # y tile per (pass, ab): col = s*6144 + (par XOR s)*3072
            #                              + kp*1024 + oh*512 + b*128 + o
            # (par XOR s) equals the out-parity po the part serves, making
            # the conv's (A,B) pair a uniform stride-6144 AP for both po.
            y_sb = {}
            for ps in range(2):
                for ab in range(NAB):
                    y_sb[ps, ab] = persist.tile(
                        [S, 12288], F8, tag=f"y{ps}{ab}",
                        name=f"y_sb{ps}{ab}")

            def xt_ap(term, b, ab, par):
                """lhsT [i_lo, 2 (ic), 128 (a)]"""
                off = ab * 256 + par * 128
                return xt_sb[term, b][:].rearrange(
                    "p (ic q) -> p ic q", ic=2)[:, :, off:off + 128]

            def mc_ap(term, c):
                """rhs [i_lo, 2 (ic), 512] — proj col chunk c of 3"""
                return mc_sb[term][:].rearrange(
                    "p (ic q) -> p ic q", ic=2)[:, :, c * 512:(c + 1) * 512]

            def y_dst(ps, ab, par, b):
                """copy dst [p, s 2, (kp,oh) 6, o 128] for proj tile
                (b, par): s=0 lands at par'=par, s=1 at par'=1-par."""
                base = y_sb[ps, ab][:]
                return bass_mod.AP(
                    base.tensor, par * 3072 + b * 128,
                    [(12288, S), (9216 - par * 6144, 2), (512, 6), (1, 128)])

            def conv_rhs(ps, po, abi, kp, oh, bp):
                """rhs [p, 2 (s), 256 (b-pair, o)]"""
                off = po * 3072 + kp * 1024 + oh * 512 + bp * 256
                return y_sb[ps, abi][:].rearrange(
                    "p (s q) -> p s q", s=2)[:, :, off:off + 256]

            def w_ap(term, po, d, kp):
                """lhsT [p, 2 (s), 128 (ar)]"""
                return w_sb[term, po, d][:].rearrange(
                    "p (s r) -> p s r", s=2)[:, :, kp * 128:(kp + 1) * 128]

            # ---- input DMAs, first-use order ----
            def xt_dma(eng, t, ic, b):
                eng.dma_start(
                    out=xt_sb[t, b][:, ic * 1024:(ic + 1) * 1024],
                    in_=xt_d[t, :, ic * 4096 + b * 1024:
                             ic * 4096 + (b + 1) * 1024])

            # Mirror the critical-path loads across the two HWDGE queues
            # (SP serves ic=0, Act serves ic=1); weights + last xt batch on
            # the gpsimd SWDGE queue.
            def mc_dma(eng, t, ic, s):
                c0 = ic * 1536 + s * 768
                eng.dma_start(out=mc_sb[t][:, c0:c0 + 768],
                              in_=mc_d[t, :, c0:c0 + 768])

            # Act queue carries NO input DMAs (its SEQ must stay free for the
            # y1 copies); SP gets only the b0-critical loads; the rest
            # streams on the independent gpsimd SWDGE path in first-use
            # order.
            xt_dma(nc.sync, 0, 0, 0)
            mc_dma(nc.sync, 0, 0, 0)
            xt_dma(nc.sync, 0, 1, 0)
            mc_dma(nc.sync, 0, 1, 0)
            xt_dma(nc.sync, 1, 0, 0)
            xt_dma(nc.sync, 1, 1, 0)
            mc_dma(nc.sync, 1, 0, 0)
            mc_dma(nc.sync, 1, 1, 0)
            for t in range(2):
                for ic in range(2):
                    mc_dma(nc.gpsimd, t, ic, 1)
            for b in range(1, 3):
                for t in range(2):
                    for ic in range(2):
                        xt_dma(nc.sync, t, ic, b)
            for t in range(2):
                for ic in range(2):
                    xt_dma(nc.gpsimd, t, ic, 3)
            for d in range(NAB):
                for po in range(2):
                    for t in range(2):
                        nc.gpsimd.dma_start(out=w_sb[t, po, d][:],
                                            in_=w_d[t, po, d])

            # ---- phases ----
            def proj_tile(ab, b, par):
                py = pyp.tile([S, 1536], F32, tag="py",
                              name=f"py_{ab}_{b}_{par}")
                for c in range(3):
                    for i_mm, (xi, mi) in enumerate(((0, 0), (1, 0), (0, 1))):
                        nc.tensor.matmul(
                            py[:, c * 512:(c + 1) * 512],
                            lhsT=xt_ap(xi, b, ab, par),
                            rhs=mc_ap(mi, c),
                            start=(i_mm == 0), stop=(i_mm == 2),
                            perf_mode=DR,
                        )
                src = py[:].rearrange("p (s m o) -> p s m o", s=2, m=6)
                nc.scalar.copy(out=y_dst(0, ab, par, b), in_=src)
                nc.vector.tensor_sub(
                    y_dst(1, ab, par, b), src, y_dst(0, ab, par, b))

            # ---- conv emission: flat mm list popped between proj s-tiles ----
            pouts = {}

            def get_pout(abo, po, oh):
                if (abo, po, oh) not in pouts:
                    pouts[abo, po, oh] = poutp.tile(
                        [S, 512], F32, tag="pout",
                        name=f"pout_{abo}_{po}_{oh}")
                return pouts[abo, po, oh]

            def mk_mm(abo, po, oh, bp, abi, kp, wt, yt, st, sp):
                def emit():
                    pout = get_pout(abo, po, oh)
                    nc.tensor.matmul(
                        pout[:, bp * 256:(bp + 1) * 256],
                        lhsT=w_ap(wt, po, abo - abi, kp),
                        rhs=conv_rhs(yt, po, abi, kp, oh, bp),
                        start=st, stop=sp, perf_mode=DR)
                return emit

            def mk_drain(abo, po, oh):
                def emit():
                    pout = pouts.pop((abo, po, oh))
                    ost = ostage.tile([S, 512], F32, tag="ost",
                                      name=f"ost_{abo}_{po}_{oh}")
                    nc.scalar.mul(ost[:], pout[:], OUT_SCALE)
                    nc.sync.dma_start(out=out_d[po, abo, oh], in_=ost[:])
                return emit

            # items: (eligible_at_stile_count, cost_us, emit_fn)
            MM_US = 0.0533
            items = []
            for abo in range(NAB):
                for po in range(2):
                    for oh in range(2):
                        for bp in range(2):
                            elig = abo * 8 + bp * 4 + 4 + 2
                            n_mm = (abo + 1) * KPC * 3
                            i_mm = 0
                            for abi in range(abo + 1):
                                for kp in range(KPC):
                                    for (wt, yt) in ((0, 0), (1, 0), (0, 1)):
                                        items.append(
                                            (elig, MM_US,
                                             mk_mm(abo, po, oh, bp, abi, kp,
                                                   wt, yt, i_mm == 0,
                                                   i_mm == n_mm - 1)))
                                        i_mm += 1
                        # drain after both bp groups stop
                        items.append((abo * 8 + 8 + 2, 0.0,
                                      mk_drain(abo, po, oh)))

            TILE_PE = 0.96
            TILE_COPY = 1.75
            ci = 0
            debt = 0.0
            tiles = 0
            for ab in range(NAB):
                for b in range(B):
                    for par in range(2):
                        proj_tile(ab, b, par)
                        tiles += 1
                        debt += TILE_COPY - TILE_PE
                        while ci < len(items) and \
                                items[ci][0] <= tiles and debt > 0:
                            items[ci][2]()
                            debt -= items[ci][1]
                            ci += 1
            while ci < len(items):
                items[ci][2]()
                ci += 1
    nc.finalize()
    return nc


def _q8(v):
    return np.asarray(v, dtype=F8NP)


def _host_pack(x, phi, M_phi_plus, M_phi_minus):
    x = np.ascontiguousarray(x, dtype=np.float32)
    phi = np.ascontiguousarray(phi, dtype=np.float32)
    Mp = np.ascontiguousarray(M_phi_plus, dtype=np.float32)
    Mm = np.ascontiguousarray(M_phi_minus, dtype=np.float32)

    # ---- xt (shared across cores) ----
    x1 = _q8(SX * x)
    x2 = _q8(SX * x - x1.astype(np.float32))
    xt = np.empty((2, S, 8192), dtype=F8NP)
    for t, xq in enumerate((x1, x2)):
        # [b, t, i] -> [i_lo, ic, b, ab, par, a]
        xr = xq.reshape(B, NAB, S, 2, 2, S)   # b, ab, a, par, ic, i_lo
        # t = 2*(ab*128+a)+par ; i = ic*128+i_lo
        xr = xr.transpose(5, 4, 0, 1, 3, 2)   # i_lo, ic, b, ab, par, a
        xt[t] = np.ascontiguousarray(xr.reshape(S, 8192))

    # ---- per-core mc ----
    A = Mp + Mm
    Bm = Mp - Mm
    mc_cores = []
    for c in range(N_CORES):
        ks = slice(KPC * c, KPC * (c + 1))
        # [s, kp, i, o]
        cm = np.stack([A[ks], Bm[ks]], axis=0).astype(np.float32)
        m1 = _q8(SM * cm)
        m2 = _q8(SM * cm - m1.astype(np.float32))
        mcc = np.empty((2, S, 3072), dtype=F8NP)
        for t, mq in enumerate((m1, m2)):
            mr = mq.reshape(2, KPC, 2, S, O)      # s, kp, ic, i_lo, o
            mr = mr.transpose(3, 2, 0, 1, 4)      # i_lo, ic, s, kp, o
            mcc[t] = np.ascontiguousarray(mr.reshape(S, 3072))
        mc_cores.append(mcc)

    # ---- per-core w ----
    ar = np.arange(S)
    br = np.arange(S)
    w_cores = []
    # fam blocks [d, br, ar, K]
    dmat = ar[None, None, :] - br[None, :, None] + (np.arange(NAB) * S)[:, None, None]

    def fam(idx, valid):
        return np.where(valid[..., None],
                        phi[np.clip(idx, 0, L - 1), :], 0.0).astype(np.float32)

    WE = fam(2 * dmat, dmat >= 0)
    WO1 = fam(2 * dmat - 1, dmat >= 1)
    WO2 = fam(2 * dmat + 1, dmat >= 0)
    for c in range(N_CORES):
        ks = slice(KPC * c, KPC * (c + 1))
        # [po, d, br, s, kp, ar]
        wc = np.empty((2, NAB, S, 2, KPC, S), dtype=np.float32)
        for po in range(2):
            fams = (WE, WO1) if po == 0 else (WE, WO2)
            for s, F in enumerate(fams):
                # F: [d, br, ar, K] -> [d, br, kp, ar]
                wc[po, :, :, s] = F[:, :, :, ks].transpose(0, 1, 3, 2)
        w1 = _q8(SW * wc)
        w2 = _q8(SW * wc - w1.astype(np.float32))
        wcc = np.empty((2, 2, NAB, S, 768), dtype=F8NP)
        for t, wq in enumerate((w1, w2)):
            wcc[t] = wq.reshape(2, NAB, S, 768)
        w_cores.append(wcc)

    return xt, mc_cores, w_cores


def kernel(x, phi, M_phi_plus, M_phi_minus):
    if "nc" not in _cache:
        _cache["nc"] = _build_program()
    nc = _cache["nc"]

    xt, mc_cores, w_cores = _host_pack(x, phi, M_phi_plus, M_phi_minus)
    in_maps = [
        {"xt": xt, "mc": mc_cores[c], "w": w_cores[c]}
        for c in range(N_CORES)
    ]
    res = None
    last_err = None
    for attempt in range(3):
        try:
            res = run_bass_kernel_spmd(nc, in_maps,
                                       core_ids=list(range(N_CORES)))
            break
        except Exception as e:
            last_err = e
    if res is None:
        raise last_err
    # out[po, ab, oh, ar, b*128+o] -> [b, l, o]; sum over cores
    acc = np.zeros((2, NAB, 2, S, B * S), dtype=np.float64)
    for om in res.results:
        acc += om["out"]
    acc = acc.reshape(2, NAB, 2, S, B, S)        # po, ab, oh, ar, b, o
    out = np.empty((B, L, O), dtype=np.float64)
    la = acc.transpose(4, 1, 3, 0, 2, 5)         # b, ab, ar, po, oh, o
    out = la.reshape(B, L // 2, 2, O)            # b, (ab ar), po, (oh o)
    out = out.reshape(B, L // 2, 2, O).transpose(0, 1, 2, 3)
    out2 = np.empty((B, L, O), dtype=np.float64)
    out2[:, 0::2, :] = out[:, :, 0, :]
    out2[:, 1::2, :] = out[:, :, 1, :]
    return np.ascontiguousarray(out2.astype(np.float32))


# revision 22
# speedup vs baseline: 1.1274x; 1.1274x over previous
"""MiniSTU (spectral transform unit) Trainium2 kernel — fp8 DoubleRow version.

Math: out[b,l,o] = sum_k sum_{t<=l} phi_k[l-t] * (x[b,t] @ C_k^{par(l-t)})[o]
where C^even = Mp+Mm =: A, C^odd = Mp-Mm =: B  (parity merge of the two
sign branches of the reference's FFT conv — halves the conv work).

Deinterleaving t and l by parity turns this into 4 half-length (512) causal
block-Toeplitz convolutions built from three filter families:
  famE[d]  = phi[2d]   (A-stream, both output parities)
  famO1[d] = phi[2d-1] (B-stream -> even outputs)
  famO2[d] = phi[2d+1] (B-stream -> odd outputs)

All matmuls run as fp8e4 (e4m3) DoubleRow (0.5 cycles/row in the cost
model) with 3-term error compensation: for X@W with X1=q(X), X2=q(X-X1),
W1=q(W), W2=q(W-W1):  X@W ~= X1W1 + X2W1 + X1W2  (dropped X2W2 ~ 0.1%).
The DoubleRow pair dim carries the two i-chunks (projection: contraction
256 per instruction) or the two streams A/B (conv: both streams' block
contributions per instruction).

Sharding: K=24 filters split 3-per-core across 8 cores; host sums the 8
partial outputs.  Scales (powers of 2): x*8, M*8, W*256; device psum
y = 64*y_true, out_psum = 16384*out_true; the 1/16384 is applied by the
on-device output copy.
"""

import os
os.environ.setdefault("NEURON_RT_RESET_CORES", "1")

import numpy as np
import ml_dtypes
import concourse.bacc as bacc
import concourse.mybir as mybir
from concourse.tile import TileContext
import concourse.bass as bass_mod
from concourse.bass_utils import run_bass_kernel_spmd

B, L, I, O, K = 4, 1024, 256, 256, 24
S = 128
NAB = 4            # deinterleaved half-length 512 = 4 blocks of 128
KPC = 3
N_CORES = 8
F32 = mybir.dt.float32
F8 = mybir.dt.float8e4
F8NP = ml_dtypes.float8_e4m3
BF16 = mybir.dt.bfloat16
DR = mybir.MatmulPerfMode.DoubleRow

SX, SM, SW = 8.0, 8.0, 256.0
OUT_SCALE = 1.0 / (SX * SM * SW)

_cache = {}


def _build_program():
    nc = bacc.Bacc()
    # xt[term][i_lo, ic*4096 + b*1024 + ab*256 + par*128 + a]
    xt_d = nc.declare_dram_parameter("xt", [2, S, 8192], F8, isOutput=False)
    # mc[term][i_lo, ic*1536 + s*768 + kp*256 + o]
    mc_d = nc.declare_dram_parameter("mc", [2, S, 3072], F8, isOutput=False)
    # w[term][po, d, br, s*384 + kp*128 + ar]
    w_d = nc.declare_dram_parameter("w", [2, 2, NAB, S, 768], F8, isOutput=False)
    # out[po, ab, oh, ar, b*128+o]
    out_d = nc.declare_dram_parameter("out", [2, NAB, 2, S, B * S], F32,
                                      isOutput=True)

    with TileContext(nc) as tc:
        with tc.tile_pool(name="persist", bufs=1) as persist, \
             tc.tile_pool(name="ostage", bufs=3) as ostage, \
             tc.tile_pool(name="pyp", bufs=3, space="PSUM") as pyp, \
             tc.tile_pool(name="poutp", bufs=2, space="PSUM") as poutp:

            # ---- persistent SBUF tiles ----
            # xt tile per (term, b): cols = ic*1024 + ab*256 + par*128 + a
            xt_sb = {}
            for t in range(2):
                for b in range(B):
                    xt_sb[t, b] = persist.tile([S, 2048], F8, tag=f"xt{t}{b}",
                                               name=f"xt_sb{t}{b}")
            mc_sb = {}
            for t in range(2):
                mc_sb[t] = persist.tile([S, 3072], F8, tag=f"mc{t}",
                                        name=f"mc_sb{t}")
            w_sb = {}
            for t in range(2):
                for po in range(2):
                    for d in range(NAB):
                        w_sb[t, po, d] = persist.tile(
                            [S, 768], F8, tag=f"w{t}{po}{d}",
                            name=f"w_sb{t}{po}{d}")
            # y tile per (pass, po, ab): cols = s*3072 + kp*1024 + oh*512 + b*128 + o
            y_sb = {}
            for ps in range(2):
                for po in range(2):
                    for ab in range(NAB):
                        y_sb[ps, po, ab] = persist.tile(
                            [S, 6144], F8, tag=f"y{ps}{po}{ab}",
                            name=f"y_sb{ps}{po}{ab}")

            def xt_ap(term, b, ab, par):
                """lhsT [i_lo, 2 (ic), 128 (a)]"""
                off = ab * 256 + par * 128
                return xt_sb[term, b][:].rearrange(
                    "p (ic q) -> p ic q", ic=2)[:, :, off:off + 128]

            def mc_ap(term, s, c0, cw):
                """rhs [i_lo, 2 (ic), cw] — cols [c0, c0+cw) of stream s"""
                base = s * 768 + c0
                return mc_sb[term][:].rearrange(
                    "p (ic q) -> p ic q", ic=2)[:, :, base:base + cw]

            def y_dst(ps, po, ab, s, b):
                """copy dst [p, (kp,oh) 6, o 128] for stream s, batch b"""
                base = y_sb[ps, po, ab][:]
                return bass_mod.AP(
                    base.tensor, s * 3072 + b * 128,
                    [(6144, S), (512, 6), (1, 128)])

            def conv_rhs(ps, po, abi, kp, oh, bp):
                """rhs [p, 2 (s), 256 (b-pair, o)]"""
                off = kp * 1024 + oh * 512 + bp * 256
                return y_sb[ps, po, abi][:].rearrange(
                    "p (s q) -> p s q", s=2)[:, :, off:off + 256]

            def w_ap(term, po, d, kp):
                """lhsT [p, 2 (s), 128 (ar)]"""
                return w_sb[term, po, d][:].rearrange(
                    "p (s r) -> p s r", s=2)[:, :, kp * 128:(kp + 1) * 128]

            # ---- input DMAs, first-use order ----
            def xt_dma(eng, t, ic, b):
                eng.dma_start(
                    out=xt_sb[t, b][:, ic * 1024:(ic + 1) * 1024],
                    in_=xt_d[t, :, ic * 4096 + b * 1024:
                             ic * 4096 + (b + 1) * 1024])

            # Mirror the critical-path loads across the two HWDGE queues
            # (SP serves ic=0, Act serves ic=1); weights + last xt batch on
            # the gpsimd SWDGE queue.
            def mc_dma(eng, t, ic, s):
                c0 = ic * 1536 + s * 768
                eng.dma_start(out=mc_sb[t][:, c0:c0 + 768],
                              in_=mc_d[t, :, c0:c0 + 768])

            # Act queue carries NO input DMAs (its SEQ must stay free for the
            # y1 copies); SP gets only the b0-critical loads; the rest
            # streams on the independent gpsimd SWDGE path in first-use
            # order.
            xt_dma(nc.sync, 0, 0, 0)
            mc_dma(nc.sync, 0, 0, 0)
            xt_dma(nc.sync, 0, 1, 0)
            mc_dma(nc.sync, 0, 1, 0)
            xt_dma(nc.sync, 1, 0, 0)
            xt_dma(nc.sync, 1, 1, 0)
            mc_dma(nc.sync, 1, 0, 0)
            mc_dma(nc.sync, 1, 1, 0)
            for t in range(2):
                for ic in range(2):
                    mc_dma(nc.gpsimd, t, ic, 1)
            for b in range(1, 3):
                for t in range(2):
                    for ic in range(2):
                        xt_dma(nc.sync, t, ic, b)
            for t in range(2):
                for ic in range(2):
                    xt_dma(nc.gpsimd, t, ic, 3)
            for d in range(NAB):
                for po in range(2):
                    for t in range(2):
                        nc.gpsimd.dma_start(out=w_sb[t, po, d][:],
                                            in_=w_d[t, po, d])

            # ---- phases ----
            def proj_stile(ab, b, par, s):
                py = pyp.tile([S, 768], F32, tag="py",
                              name=f"py_{ab}_{b}_{par}_{s}")
                i_mm = 0
                for c0, cw in ((0, 512), (512, 256)):
                    for (xi, mi) in ((0, 0), (1, 0), (0, 1)):
                        nc.tensor.matmul(
                            py[:, c0:c0 + cw],
                            lhsT=xt_ap(xi, b, ab, par),
                            rhs=mc_ap(mi, s, c0, cw),
                            start=(i_mm in (0, 3)), stop=(i_mm in (2, 5)),
                            perf_mode=DR,
                        )
                        i_mm += 1
                po = par if s == 0 else 1 - par
                src = py[:].rearrange("p (m o) -> p m o", m=6)
                nc.scalar.copy(out=y_dst(0, po, ab, s, b), in_=src)
                nc.vector.tensor_sub(
                    y_dst(1, po, ab, s, b), src, y_dst(0, po, ab, s, b))

            # ---- conv emission: flat mm list popped between proj s-tiles ----
            pouts = {}

            def get_pout(abo, po, oh):
                if (abo, po, oh) not in pouts:
                    pouts[abo, po, oh] = poutp.tile(
                        [S, 512], F32, tag="pout",
                        name=f"pout_{abo}_{po}_{oh}")
                return pouts[abo, po, oh]

            def mk_mm(abo, po, oh, bp, abi, kp, wt, yt, st, sp):
                def emit():
                    pout = get_pout(abo, po, oh)
                    nc.tensor.matmul(
                        pout[:, bp * 256:(bp + 1) * 256],
                        lhsT=w_ap(wt, po, abo - abi, kp),
                        rhs=conv_rhs(yt, po, abi, kp, oh, bp),
                        start=st, stop=sp, perf_mode=DR)
                return emit

            def mk_drain(abo, po, oh):
                def emit():
                    pout = pouts.pop((abo, po, oh))
                    ost = ostage.tile([S, 512], F32, tag="ost",
                                      name=f"ost_{abo}_{po}_{oh}")
                    nc.scalar.mul(ost[:], pout[:], OUT_SCALE)
                    nc.sync.dma_start(out=out_d[po, abo, oh], in_=ost[:])
                return emit

            # items: (eligible_at_stile_count, cost_us, emit_fn)
            MM_US = 0.0533
            items = []
            for abo in range(NAB):
                for po in range(2):
                    for oh in range(2):
                        for bp in range(2):
                            elig = (abo * 8 + bp * 4 + 4) * 2 + 2
                            n_mm = (abo + 1) * KPC * 3
                            i_mm = 0
                            for abi in range(abo + 1):
                                for kp in range(KPC):
                                    for (wt, yt) in ((0, 0), (1, 0), (0, 1)):
                                        items.append(
                                            (elig, MM_US,
                                             mk_mm(abo, po, oh, bp, abi, kp,
                                                   wt, yt, i_mm == 0,
                                                   i_mm == n_mm - 1)))
                                        i_mm += 1
                        # drain after both bp groups stop
                        items.append(((abo * 8 + 8) * 2 + 4, 0.0,
                                      mk_drain(abo, po, oh)))

            STILE_PE = 0.48
            STILE_COPY = 0.93
            ci = 0
            debt = 0.0
            stiles = 0
            for ab in range(NAB):
                for b in range(B):
                    for par in range(2):
                        for s in range(2):
                            proj_stile(ab, b, par, s)
                            stiles += 1
                            debt += STILE_COPY - STILE_PE
                            while ci < len(items) and \
                                    items[ci][0] <= stiles and debt > 0:
                                items[ci][2]()
                                debt -= items[ci][1]
                                ci += 1
            while ci < len(items):
                items[ci][2]()
                ci += 1
    nc.finalize()
    return nc


def _q8(v):
    return np.asarray(v, dtype=F8NP)


def _host_pack(x, phi, M_phi_plus, M_phi_minus):
    x = np.ascontiguousarray(x, dtype=np.float32)
    phi = np.ascontiguousarray(phi, dtype=np.float32)
    Mp = np.ascontiguousarray(M_phi_plus, dtype=np.float32)
    Mm = np.ascontiguousarray(M_phi_minus, dtype=np.float32)

    # ---- xt (shared across cores) ----
    x1 = _q8(SX * x)
    x2 = _q8(SX * x - x1.astype(np.float32))
    xt = np.empty((2, S, 8192), dtype=F8NP)
    for t, xq in enumerate((x1, x2)):
        # [b, t, i] -> [i_lo, ic, b, ab, par, a]
        xr = xq.reshape(B, NAB, S, 2, 2, S)   # b, ab, a, par, ic, i_lo
        # t = 2*(ab*128+a)+par ; i = ic*128+i_lo
        xr = xr.transpose(5, 4, 0, 1, 3, 2)   # i_lo, ic, b, ab, par, a
        xt[t] = np.ascontiguousarray(xr.reshape(S, 8192))

    # ---- per-core mc ----
    A = Mp + Mm
    Bm = Mp - Mm
    mc_cores = []
    for c in range(N_CORES):
        ks = slice(KPC * c, KPC * (c + 1))
        # [s, kp, i, o]
        cm = np.stack([A[ks], Bm[ks]], axis=0).astype(np.float32)
        m1 = _q8(SM * cm)
        m2 = _q8(SM * cm - m1.astype(np.float32))
        mcc = np.empty((2, S, 3072), dtype=F8NP)
        for t, mq in enumerate((m1, m2)):
            mr = mq.reshape(2, KPC, 2, S, O)      # s, kp, ic, i_lo, o
            mr = mr.transpose(3, 2, 0, 1, 4)      # i_lo, ic, s, kp, o
            mcc[t] = np.ascontiguousarray(mr.reshape(S, 3072))
        mc_cores.append(mcc)

    # ---- per-core w ----
    ar = np.arange(S)
    br = np.arange(S)
    w_cores = []
    # fam blocks [d, br, ar, K]
    dmat = ar[None, None, :] - br[None, :, None] + (np.arange(NAB) * S)[:, None, None]

    def fam(idx, valid):
        return np.where(valid[..., None],
                        phi[np.clip(idx, 0, L - 1), :], 0.0).astype(np.float32)

    WE = fam(2 * dmat, dmat >= 0)
    WO1 = fam(2 * dmat - 1, dmat >= 1)
    WO2 = fam(2 * dmat + 1, dmat >= 0)
    for c in range(N_CORES):
        ks = slice(KPC * c, KPC * (c + 1))
        # [po, d, br, s, kp, ar]
        wc = np.empty((2, NAB, S, 2, KPC, S), dtype=np.float32)
        for po in range(2):
            fams = (WE, WO1) if po == 0 else (WE, WO2)
            for s, F in enumerate(fams):
                # F: [d, br, ar, K] -> [d, br, kp, ar]
                wc[po, :, :, s] = F[:, :, :, ks].transpose(0, 1, 3, 2)
        w1 = _q8(SW * wc)
        w2 = _q8(SW * wc - w1.astype(np.float32))
        wcc = np.empty((2, 2, NAB, S, 768), dtype=F8NP)
        for t, wq in enumerate((w1, w2)):
            wcc[t] = wq.reshape(2, NAB, S, 768)
        w_cores.append(wcc)

    return xt, mc_cores, w_cores


def kernel(x, phi, M_phi_plus, M_phi_minus):
    if "nc" not in _cache:
        _cache["nc"] = _build_program()
    nc = _cache["nc"]

    xt, mc_cores, w_cores = _host_pack(x, phi, M_phi_plus, M_phi_minus)
    in_maps = [
        {"xt": xt, "mc": mc_cores[c], "w": w_cores[c]}
        for c in range(N_CORES)
    ]
    res = None
    last_err = None
    for attempt in range(3):
        try:
            res = run_bass_kernel_spmd(nc, in_maps,
                                       core_ids=list(range(N_CORES)))
            break
        except Exception as e:
            last_err = e
    if res is None:
        raise last_err
    # out[po, ab, oh, ar, b*128+o] -> [b, l, o]; sum over cores
    acc = np.zeros((2, NAB, 2, S, B * S), dtype=np.float64)
    for om in res.results:
        acc += om["out"]
    acc = acc.reshape(2, NAB, 2, S, B, S)        # po, ab, oh, ar, b, o
    out = np.empty((B, L, O), dtype=np.float64)
    la = acc.transpose(4, 1, 3, 0, 2, 5)         # b, ab, ar, po, oh, o
    out = la.reshape(B, L // 2, 2, O)            # b, (ab ar), po, (oh o)
    out = out.reshape(B, L // 2, 2, O).transpose(0, 1, 2, 3)
    out2 = np.empty((B, L, O), dtype=np.float64)
    out2[:, 0::2, :] = out[:, :, 0, :]
    out2[:, 1::2, :] = out[:, :, 1, :]
    return np.ascontiguousarray(out2.astype(np.float32))


# revision 23
# speedup vs baseline: 1.1318x; 1.0038x over previous
"""MiniSTU (spectral transform unit) Trainium2 kernel — fp8 DoubleRow version.

Math: out[b,l,o] = sum_k sum_{t<=l} phi_k[l-t] * (x[b,t] @ C_k^{par(l-t)})[o]
where C^even = Mp+Mm =: A, C^odd = Mp-Mm =: B  (parity merge of the two
sign branches of the reference's FFT conv — halves the conv work).

Deinterleaving t and l by parity turns this into 4 half-length (512) causal
block-Toeplitz convolutions built from three filter families:
  famE[d]  = phi[2d]   (A-stream, both output parities)
  famO1[d] = phi[2d-1] (B-stream -> even outputs)
  famO2[d] = phi[2d+1] (B-stream -> odd outputs)

All matmuls run as fp8e4 (e4m3) DoubleRow (0.5 cycles/row in the cost
model) with 3-term error compensation: for X@W with X1=q(X), X2=q(X-X1),
W1=q(W), W2=q(W-W1):  X@W ~= X1W1 + X2W1 + X1W2  (dropped X2W2 ~ 0.1%).
The DoubleRow pair dim carries the two i-chunks (projection: contraction
256 per instruction) or the two streams A/B (conv: both streams' block
contributions per instruction).

Sharding: K=24 filters split 3-per-core across 8 cores; host sums the 8
partial outputs.  Scales (powers of 2): x*8, M*8, W*256; device psum
y = 64*y_true, out_psum = 16384*out_true; the 1/16384 is applied by the
on-device output copy.
"""

import os
os.environ.setdefault("NEURON_RT_RESET_CORES", "1")

import numpy as np
import ml_dtypes
import concourse.bacc as bacc
import concourse.mybir as mybir
from concourse.tile import TileContext
import concourse.bass as bass_mod
from concourse.bass_utils import run_bass_kernel_spmd

B, L, I, O, K = 4, 1024, 256, 256, 24
S = 128
NAB = 4            # deinterleaved half-length 512 = 4 blocks of 128
KPC = 3
N_CORES = 8
F32 = mybir.dt.float32
F8 = mybir.dt.float8e4
F8NP = ml_dtypes.float8_e4m3
BF16 = mybir.dt.bfloat16
DR = mybir.MatmulPerfMode.DoubleRow

SX, SM, SW = 8.0, 8.0, 256.0
OUT_SCALE = 1.0 / (SX * SM * SW)

_cache = {}


def _build_program():
    nc = bacc.Bacc()
    # xt[term][i_lo, ic*4096 + b*1024 + ab*256 + par*128 + a]
    xt_d = nc.declare_dram_parameter("xt", [2, S, 8192], F8, isOutput=False)
    # mc[term][i_lo, ic*1536 + s*768 + kp*256 + o]
    mc_d = nc.declare_dram_parameter("mc", [2, S, 3072], F8, isOutput=False)
    # w[term][po, d, br, s*384 + kp*128 + ar]
    w_d = nc.declare_dram_parameter("w", [2, 2, NAB, S, 768], F8, isOutput=False)
    # out[po, ab, oh, ar, b*128+o]
    out_d = nc.declare_dram_parameter("out", [2, NAB, 2, S, B * S], BF16,
                                      isOutput=True)

    with TileContext(nc) as tc:
        with tc.tile_pool(name="persist", bufs=1) as persist, \
             tc.tile_pool(name="ostage", bufs=3) as ostage, \
             tc.tile_pool(name="pyp", bufs=3, space="PSUM") as pyp, \
             tc.tile_pool(name="poutp", bufs=2, space="PSUM") as poutp:

            # ---- persistent SBUF tiles ----
            # xt tile per (term, b): cols = ic*1024 + ab*256 + par*128 + a
            xt_sb = {}
            for t in range(2):
                for b in range(B):
                    xt_sb[t, b] = persist.tile([S, 2048], F8, tag=f"xt{t}{b}",
                                               name=f"xt_sb{t}{b}")
            mc_sb = {}
            for t in range(2):
                mc_sb[t] = persist.tile([S, 3072], F8, tag=f"mc{t}",
                                        name=f"mc_sb{t}")
            w_sb = {}
            for t in range(2):
                for po in range(2):
                    for d in range(NAB):
                        w_sb[t, po, d] = persist.tile(
                            [S, 768], F8, tag=f"w{t}{po}{d}",
                            name=f"w_sb{t}{po}{d}")
            # y tile per (pass, po, ab): cols = s*3072 + kp*1024 + oh*512 + b*128 + o
            y_sb = {}
            for ps in range(2):
                for po in range(2):
                    for ab in range(NAB):
                        y_sb[ps, po, ab] = persist.tile(
                            [S, 6144], F8, tag=f"y{ps}{po}{ab}",
                            name=f"y_sb{ps}{po}{ab}")

            def xt_ap(term, b, ab, par):
                """lhsT [i_lo, 2 (ic), 128 (a)]"""
                off = ab * 256 + par * 128
                return xt_sb[term, b][:].rearrange(
                    "p (ic q) -> p ic q", ic=2)[:, :, off:off + 128]

            def mc_ap(term, s, c0, cw):
                """rhs [i_lo, 2 (ic), cw] — cols [c0, c0+cw) of stream s"""
                base = s * 768 + c0
                return mc_sb[term][:].rearrange(
                    "p (ic q) -> p ic q", ic=2)[:, :, base:base + cw]

            def y_dst(ps, po, ab, s, b):
                """copy dst [p, (kp,oh) 6, o 128] for stream s, batch b"""
                base = y_sb[ps, po, ab][:]
                return bass_mod.AP(
                    base.tensor, s * 3072 + b * 128,
                    [(6144, S), (512, 6), (1, 128)])

            def conv_rhs(ps, po, abi, kp, oh, bp):
                """rhs [p, 2 (s), 256 (b-pair, o)]"""
                off = kp * 1024 + oh * 512 + bp * 256
                return y_sb[ps, po, abi][:].rearrange(
                    "p (s q) -> p s q", s=2)[:, :, off:off + 256]

            def w_ap(term, po, d, kp):
                """lhsT [p, 2 (s), 128 (ar)]"""
                return w_sb[term, po, d][:].rearrange(
                    "p (s r) -> p s r", s=2)[:, :, kp * 128:(kp + 1) * 128]

            # Preload the Act engine's Copy activation table during the
            # initial DMA wait so the first y1 copy doesn't pay the
            # 1.3us table load.
            scr = persist.tile([S, 2], F32, tag="scr", name="scr_sb")
            nc.vector.memset(scr[:], 0.0)
            nc.scalar.copy(out=scr[:, 1:2], in_=scr[:, 0:1])

            # ---- input DMAs, first-use order ----
            def xt_dma(eng, t, ic, b):
                eng.dma_start(
                    out=xt_sb[t, b][:, ic * 1024:(ic + 1) * 1024],
                    in_=xt_d[t, :, ic * 4096 + b * 1024:
                             ic * 4096 + (b + 1) * 1024])

            # Mirror the critical-path loads across the two HWDGE queues
            # (SP serves ic=0, Act serves ic=1); weights + last xt batch on
            # the gpsimd SWDGE queue.
            def mc_dma(eng, t, ic, s):
                c0 = ic * 1536 + s * 768
                eng.dma_start(out=mc_sb[t][:, c0:c0 + 768],
                              in_=mc_d[t, :, c0:c0 + 768])

            # Act queue carries NO input DMAs (its SEQ must stay free for the
            # y1 copies); SP gets only the b0-critical loads; the rest
            # streams on the independent gpsimd SWDGE path in first-use
            # order.
            xt_dma(nc.sync, 0, 0, 0)
            xt_dma(nc.sync, 0, 1, 0)
            mc_dma(nc.sync, 0, 0, 0)
            mc_dma(nc.sync, 0, 1, 0)
            xt_dma(nc.sync, 1, 0, 0)
            xt_dma(nc.sync, 1, 1, 0)
            mc_dma(nc.sync, 1, 0, 0)
            mc_dma(nc.sync, 1, 1, 0)
            for t in range(2):
                for ic in range(2):
                    mc_dma(nc.gpsimd, t, ic, 1)
            for b in range(1, 3):
                for t in range(2):
                    for ic in range(2):
                        xt_dma(nc.sync, t, ic, b)
            for t in range(2):
                for ic in range(2):
                    xt_dma(nc.gpsimd, t, ic, 3)
            for d in range(NAB):
                for po in range(2):
                    for t in range(2):
                        nc.gpsimd.dma_start(out=w_sb[t, po, d][:],
                                            in_=w_d[t, po, d])

            # ---- phases ----
            def proj_stile(ab, b, par, s):
                py = pyp.tile([S, 768], F32, tag="py",
                              name=f"py_{ab}_{b}_{par}_{s}")
                i_mm = 0
                for c0, cw in ((0, 512), (512, 256)):
                    for (xi, mi) in ((0, 0), (1, 0), (0, 1)):
                        nc.tensor.matmul(
                            py[:, c0:c0 + cw],
                            lhsT=xt_ap(xi, b, ab, par),
                            rhs=mc_ap(mi, s, c0, cw),
                            start=(i_mm in (0, 3)), stop=(i_mm in (2, 5)),
                            perf_mode=DR,
                        )
                        i_mm += 1
                po = par if s == 0 else 1 - par
                src = py[:].rearrange("p (m o) -> p m o", m=6)
                nc.scalar.copy(out=y_dst(0, po, ab, s, b), in_=src)
                nc.vector.tensor_sub(
                    y_dst(1, po, ab, s, b), src, y_dst(0, po, ab, s, b))

            # ---- conv emission: flat mm list popped between proj s-tiles ----
            pouts = {}

            def get_pout(abo, po, oh):
                if (abo, po, oh) not in pouts:
                    pouts[abo, po, oh] = poutp.tile(
                        [S, 512], F32, tag="pout",
                        name=f"pout_{abo}_{po}_{oh}")
                return pouts[abo, po, oh]

            def mk_mm(abo, po, oh, bp, abi, kp, wt, yt, st, sp):
                def emit():
                    pout = get_pout(abo, po, oh)
                    nc.tensor.matmul(
                        pout[:, bp * 256:(bp + 1) * 256],
                        lhsT=w_ap(wt, po, abo - abi, kp),
                        rhs=conv_rhs(yt, po, abi, kp, oh, bp),
                        start=st, stop=sp, perf_mode=DR)
                return emit

            def mk_drain(abo, po, oh):
                def emit():
                    pout = pouts.pop((abo, po, oh))
                    ost = ostage.tile([S, 512], BF16, tag="ost",
                                      name=f"ost_{abo}_{po}_{oh}")
                    nc.scalar.mul(ost[:], pout[:], OUT_SCALE)
                    nc.sync.dma_start(out=out_d[po, abo, oh], in_=ost[:])
                return emit

            # items: (eligible_at_stile_count, cost_us, emit_fn)
            MM_US = 0.0533
            items = []
            for abo in range(NAB):
                for po in range(2):
                    for oh in range(2):
                        for bp in range(2):
                            elig = (abo * 8 + bp * 4 + 4) * 2 + 2
                            n_mm = (abo + 1) * KPC * 3
                            i_mm = 0
                            for abi in range(abo + 1):
                                for kp in range(KPC):
                                    for (wt, yt) in ((0, 0), (1, 0), (0, 1)):
                                        items.append(
                                            (elig, MM_US,
                                             mk_mm(abo, po, oh, bp, abi, kp,
                                                   wt, yt, i_mm == 0,
                                                   i_mm == n_mm - 1)))
                                        i_mm += 1
                        # drain after both bp groups stop
                        items.append(((abo * 8 + 8) * 2 + 4, 0.0,
                                      mk_drain(abo, po, oh)))

            STILE_PE = 0.48
            STILE_COPY = 0.93
            ci = 0
            debt = 0.0
            stiles = 0
            for ab in range(NAB):
                for b in range(B):
                    for par in range(2):
                        for s in range(2):
                            proj_stile(ab, b, par, s)
                            stiles += 1
                            debt += STILE_COPY - STILE_PE
                            while ci < len(items) and \
                                    items[ci][0] <= stiles and debt > 0:
                                items[ci][2]()
                                debt -= items[ci][1]
                                ci += 1
            while ci < len(items):
                items[ci][2]()
                ci += 1
    nc.finalize()
    return nc


def _q8(v):
    return np.asarray(v, dtype=F8NP)


def _host_pack(x, phi, M_phi_plus, M_phi_minus):
    x = np.ascontiguousarray(x, dtype=np.float32)
    phi = np.ascontiguousarray(phi, dtype=np.float32)
    Mp = np.ascontiguousarray(M_phi_plus, dtype=np.float32)
    Mm = np.ascontiguousarray(M_phi_minus, dtype=np.float32)

    # ---- xt (shared across cores) ----
    x1 = _q8(SX * x)
    x2 = _q8(SX * x - x1.astype(np.float32))
    xt = np.empty((2, S, 8192), dtype=F8NP)
    for t, xq in enumerate((x1, x2)):
        # [b, t, i] -> [i_lo, ic, b, ab, par, a]
        xr = xq.reshape(B, NAB, S, 2, 2, S)   # b, ab, a, par, ic, i_lo
        # t = 2*(ab*128+a)+par ; i = ic*128+i_lo
        xr = xr.transpose(5, 4, 0, 1, 3, 2)   # i_lo, ic, b, ab, par, a
        xt[t] = np.ascontiguousarray(xr.reshape(S, 8192))

    # ---- per-core mc ----
    A = Mp + Mm
    Bm = Mp - Mm
    mc_cores = []
    for c in range(N_CORES):
        ks = slice(KPC * c, KPC * (c + 1))
        # [s, kp, i, o]
        cm = np.stack([A[ks], Bm[ks]], axis=0).astype(np.float32)
        m1 = _q8(SM * cm)
        m2 = _q8(SM * cm - m1.astype(np.float32))
        mcc = np.empty((2, S, 3072), dtype=F8NP)
        for t, mq in enumerate((m1, m2)):
            mr = mq.reshape(2, KPC, 2, S, O)      # s, kp, ic, i_lo, o
            mr = mr.transpose(3, 2, 0, 1, 4)      # i_lo, ic, s, kp, o
            mcc[t] = np.ascontiguousarray(mr.reshape(S, 3072))
        mc_cores.append(mcc)

    # ---- per-core w ----
    ar = np.arange(S)
    br = np.arange(S)
    w_cores = []
    # fam blocks [d, br, ar, K]
    dmat = ar[None, None, :] - br[None, :, None] + (np.arange(NAB) * S)[:, None, None]

    def fam(idx, valid):
        return np.where(valid[..., None],
                        phi[np.clip(idx, 0, L - 1), :], 0.0).astype(np.float32)

    WE = fam(2 * dmat, dmat >= 0)
    WO1 = fam(2 * dmat - 1, dmat >= 1)
    WO2 = fam(2 * dmat + 1, dmat >= 0)
    for c in range(N_CORES):
        ks = slice(KPC * c, KPC * (c + 1))
        # [po, d, br, s, kp, ar]
        wc = np.empty((2, NAB, S, 2, KPC, S), dtype=np.float32)
        for po in range(2):
            fams = (WE, WO1) if po == 0 else (WE, WO2)
            for s, F in enumerate(fams):
                # F: [d, br, ar, K] -> [d, br, kp, ar]
                wc[po, :, :, s] = F[:, :, :, ks].transpose(0, 1, 3, 2)
        w1 = _q8(SW * wc)
        w2 = _q8(SW * wc - w1.astype(np.float32))
        wcc = np.empty((2, 2, NAB, S, 768), dtype=F8NP)
        for t, wq in enumerate((w1, w2)):
            wcc[t] = wq.reshape(2, NAB, S, 768)
        w_cores.append(wcc)

    return xt, mc_cores, w_cores


def kernel(x, phi, M_phi_plus, M_phi_minus):
    if "nc" not in _cache:
        _cache["nc"] = _build_program()
    nc = _cache["nc"]

    xt, mc_cores, w_cores = _host_pack(x, phi, M_phi_plus, M_phi_minus)
    in_maps = [
        {"xt": xt, "mc": mc_cores[c], "w": w_cores[c]}
        for c in range(N_CORES)
    ]
    res = None
    last_err = None
    for attempt in range(3):
        try:
            res = run_bass_kernel_spmd(nc, in_maps,
                                       core_ids=list(range(N_CORES)))
            break
        except Exception as e:
            last_err = e
    if res is None:
        raise last_err
    # out[po, ab, oh, ar, b*128+o] -> [b, l, o]; sum over cores
    acc = np.zeros((2, NAB, 2, S, B * S), dtype=np.float64)
    for om in res.results:
        acc += om["out"]
    acc = acc.reshape(2, NAB, 2, S, B, S)        # po, ab, oh, ar, b, o
    out = np.empty((B, L, O), dtype=np.float64)
    la = acc.transpose(4, 1, 3, 0, 2, 5)         # b, ab, ar, po, oh, o
    out = la.reshape(B, L // 2, 2, O)            # b, (ab ar), po, (oh o)
    out = out.reshape(B, L // 2, 2, O).transpose(0, 1, 2, 3)
    out2 = np.empty((B, L, O), dtype=np.float64)
    out2[:, 0::2, :] = out[:, :, 0, :]
    out2[:, 1::2, :] = out[:, :, 1, :]
    return np.ascontiguousarray(out2.astype(np.float32))
